# revision 20
# baseline (speedup 1.0000x reference)
"""NeuralCA Trainium2 kernel: 64 steps of (3x3 conv 16->128, ReLU, 1x1 conv
128->16, residual, per-channel clamp) on a (8,16,256,256) state.

Sharding: pure data parallel, one batch image per NeuronCore (8 cores).

Per-core layout: the 256-row image is split into 4 slabs of 64 rows living on
partition quadrants (partition = channel + 32*slab; channels use 16 of each
32-partition quadrant). Each slab row is stored padded to 258 columns (zero
pad col on each side) plus a halo row above/below, so every 3x3 tap of the
perceive conv is a K=16 matmul on a shifted AP of the same buffer. The four
slabs' tap-matmuls target distinct PE row-groups (tile_position=(32s,0)) and
run concurrently in the 128x128 array. The 1x1 update conv is four col-tiled
M=16 bf16 matmuls (tile_position=(0,32s)) into one PSUM bank, so the
residual-add + clamp runs as two [112,512] DVE ops. Matmul data is float32r
(full-rate fp32 path, ~12 mantissa bits); h accumulates in fp32 storage.
"""
import sys

sys.path.insert(0, "/opt/trn_rl_repo")

import numpy as np
from contextlib import ExitStack

import jax
import numpy as _np
from jax.experimental.shard_map import shard_map
from jax.sharding import Mesh, PartitionSpec

import concourse.bass as bass
import concourse.bacc as bacc
import concourse.mybir as mybir
import concourse.tile as tile
from concourse import bass2jax

F32 = mybir.dt.float32
F32R = mybir.dt.float32r
BF16 = mybir.dt.bfloat16

B, C, H, W = 8, 16, 256, 256
CO = 128          # perceive output channels
STEPS = 64
SLABS = 4
SLAB_ROWS = H // SLABS          # 64
RT = SLAB_ROWS + 2              # rows per slab incl halo rows (66)
WP = W + 2                      # padded row width (258)
GENS = SLAB_ROWS // 2           # 32 generations of 2 rows each
N = 2 * W                       # matmul free size (512)

_CACHE = {}


def _build_nc():
    nc = bacc.Bacc(None, target_bir_lowering=False)
    x_in = nc.declare_dram_parameter("x", [C, H, W], F32, isOutput=False)
    wp_in = nc.declare_dram_parameter("wp", [C, 9, CO], F32, isOutput=False)
    wu_in = nc.declare_dram_parameter("wu", [CO, C], F32, isOutput=False)
    mx_in = nc.declare_dram_parameter("maxv", [128, 1], F32, isOutput=False)
    mn_in = nc.declare_dram_parameter("minv", [128, 1], F32, isOutput=False)
    y_out = nc.declare_dram_parameter("y", [C, H, W], F32, isOutput=True)

    with tile.TileContext(nc) as tc, ExitStack() as ctx:
        sb = ctx.enter_context(tc.tile_pool(name="sb", bufs=1))
        rp_pool = ctx.enter_context(tc.tile_pool(name="rp", bufs=3))
        ps_p = ctx.enter_context(
            tc.tile_pool(name="psp", bufs=6, space=bass.MemorySpace.PSUM))
        ps_dx = ctx.enter_context(
            tc.tile_pool(name="psdx", bufs=2, space=bass.MemorySpace.PSUM))

        h = sb.tile([128, RT, WP], F32)
        wp_t = sb.tile([128, 9, CO], F32)
        wu_t = sb.tile([128, C], BF16)
        mx = sb.tile([128, 1], F32)
        mn = sb.tile([128, 1], F32)
        stage = sb.tile([128, SLAB_ROWS, W], F32)
        wstage = sb.tile([128, 9 * CO + C], F32)

        # ---- init: load weights (replicated per quadrant), clamp vecs, x ----
        nc.vector.memset(stage[:], 0.0)
        # zero the parts of h that must stay zero (memset can't write f32r):
        # pad columns of every row, top halo of slab 0, bottom halo of slab 3
        zsrc = stage[:].rearrange("p a b -> p (a b)")
        for col in (0, W + 1):
            nc.vector.tensor_copy(
                out=h[:, :, col:col + 1].bitcast(F32R),
                in_=zsrc[:, :RT].rearrange("p (a b) -> p a b", b=1))
        nc.vector.tensor_copy(
            out=h[0:16, 0, :].bitcast(F32R), in_=zsrc[0:16, :WP])
        nc.vector.tensor_copy(
            out=h[96:112, 65, :].bitcast(F32R), in_=zsrc[96:112, :WP])
        for s in range(4):
            q = slice(32 * s, 32 * s + 16)
            nc.gpsimd.dma_start(
                out=wstage[q, : 9 * CO].rearrange("p (t c) -> p t c", t=9),
                in_=wp_in[:])
            nc.gpsimd.dma_start(out=stage[q], in_=x_in[:, 64 * s:64 * s + 64, :])
        nc.gpsimd.dma_start(
            out=wstage[:, 9 * CO:], in_=wu_in[:])
        nc.gpsimd.dma_start(out=mx[:], in_=mx_in[:])
        nc.gpsimd.dma_start(out=mn[:], in_=mn_in[:])

        for s in range(4):
            q = slice(32 * s, 32 * s + 16)
            nc.vector.tensor_copy(
                out=wp_t[q].bitcast(F32R),
                in_=wstage[q, : 9 * CO].rearrange("p (t c) -> p t c", t=9))
        nc.scalar.copy(out=wu_t[:], in_=wstage[:, 9 * CO:])
        # rounded image load into the interior of each slab
        nc.vector.tensor_copy(out=h[:, 1:65, 1:257].bitcast(F32R), in_=stage[:])
        # initial halo rows (also rounded data, DMA just moves bits)
        for s in range(3):
            # bottom halo of slab s := first row of slab s+1
            nc.gpsimd.dma_start(
                out=h[32 * s:32 * s + 16, 65, :].bitcast(F32R),
                in_=h[32 * s + 32:32 * s + 48, 1, :].bitcast(F32R))
            # top halo of slab s+1 := last row of slab s
            nc.gpsimd.dma_start(
                out=h[32 * s + 32:32 * s + 48, 0, :].bitcast(F32R),
                in_=h[32 * s:32 * s + 16, 64, :].bitcast(F32R))

        def emit_update_resid(r, rp_tiles):
            """1x1 conv + residual + clamp for generation r (rows 1+2r..2+2r)."""
            dxp = ps_dx.tile([128, N], F32, tag="dx")
            for s in range(4):
                nc.tensor.matmul(
                    dxp[32 * s:32 * s + 16, :],
                    wu_t[:], rp_tiles[s][:],
                    start=True, stop=True,
                    tile_position=(0, 32 * s),
                )
            rows = slice(1 + 2 * r, 3 + 2 * r)
            nc.vector.tensor_tensor(
                out=h[0:112, rows, 1:257].bitcast(F32R),
                in0=dxp[0:112].rearrange("p (a b) -> p a b", a=2),
                in1=h[0:112, rows, 1:257],
                op=mybir.AluOpType.add)
            nc.vector.tensor_scalar(
                out=h[0:112, rows, 1:257].bitcast(F32R),
                in0=h[0:112, rows, 1:257],
                scalar1=mx[0:112], scalar2=mn[0:112],
                op0=mybir.AluOpType.min, op1=mybir.AluOpType.max)

        def emit_step():
            # bottom halos must capture h_t's first slab rows BEFORE the
            # in-place residual of gen 0 overwrites them; gen 31 reads them.
            for s in range(3):
                nc.gpsimd.dma_start(
                    out=h[32 * s:32 * s + 16, 65, :].bitcast(F32R),
                    in_=h[32 * s + 32:32 * s + 48, 1, :].bitcast(F32R))
            prev = None  # relu tiles of generation r-1
            for r in range(GENS):
                # ---- perceive: 9 taps x 4 slabs, row-tiled K=16 matmuls ----
                p_tiles = []
                for s in range(4):
                    p = ps_p.tile([128, N], F32, tag="p")
                    p_tiles.append(p)
                    q = slice(32 * s, 32 * s + 16)
                    for tap in range(9):
                        dy, dx = tap // 3 - 1, tap % 3 - 1
                        rhs = h[q, 1 + 2 * r + dy:3 + 2 * r + dy, 1 + dx:257 + dx]
                        nc.tensor.matmul(
                            p[:], wp_t[q, tap, :].bitcast(F32R), rhs.bitcast(F32R),
                            start=(tap == 0), stop=(tap == 8),
                            tile_position=(32 * s, 0),
                        )
                # software pipeline: update+residual of r-1 lands after burst r
                if prev is not None:
                    emit_update_resid(r - 1, prev)
                # ---- relu + cast to bf16 (split across ACT and DVE) ----
                rp_tiles = []
                for s in range(4):
                    rp = rp_pool.tile([128, N], BF16, tag=f"rp{s % 2}")
                    rp_tiles.append(rp)
                    if s < 2:
                        nc.scalar.activation(
                            rp[:], p_tiles[s][:],
                            mybir.ActivationFunctionType.Relu)
                    else:
                        nc.vector.tensor_scalar_max(rp[:], p_tiles[s][:], 0.0)
                prev = rp_tiles
            emit_update_resid(GENS - 1, prev)
            # last gen's rows final: refresh top halos (read next step's gen 0)
            for s in range(3):
                nc.gpsimd.dma_start(
                    out=h[32 * s + 32:32 * s + 48, 0, :].bitcast(F32R),
                    in_=h[32 * s:32 * s + 16, 64, :].bitcast(F32R))

        with tc.For_i(0, STEPS, hint_engines=(mybir.EngineType.PE,
                                              mybir.EngineType.DVE)):
            emit_step()

        # ---- store result ----
        for s in range(4):
            nc.gpsimd.dma_start(
                out=y_out[:, 64 * s:64 * s + 64, :],
                in_=h[32 * s:32 * s + 16, 1:65, 1:257])
    nc.compile()
    return nc


def _get_runner():
    """Compile once; return a callable(in_maps) -> list of per-core out dicts.

    Mirrors bass2jax.run_bass_via_pjrt's multi-core path, but caches the
    jitted executable so repeated kernel() calls don't recompile the NEFF.
    """
    if "runner" in _CACHE:
        return _CACHE["runner"]
    bass2jax.install_neuronx_cc_hook()
    nc = _build_nc()

    partition_name = (nc.partition_id_tensor.name
                      if nc.partition_id_tensor else None)
    in_names, out_names, out_avals = [], [], []
    for alloc in nc.m.functions[0].allocations:
        if not isinstance(alloc, mybir.MemoryLocationSet):
            continue
        name = alloc.memorylocations[0].name
        if alloc.kind == "ExternalInput":
            if name != partition_name:
                in_names.append(name)
        elif alloc.kind == "ExternalOutput":
            out_names.append(name)
            out_avals.append(jax.core.ShapedArray(
                tuple(alloc.tensor_shape), mybir.dt.np(alloc.dtype)))
    n_params = len(in_names)
    n_outs = len(out_avals)
    all_in_names = in_names + out_names + (
        [partition_name] if partition_name else [])

    def _body(*args):
        operands = list(args)
        if partition_name is not None:
            operands.append(bass2jax.partition_id_tensor())
        return tuple(bass2jax._bass_exec_p.bind(
            *operands,
            out_avals=tuple(out_avals),
            in_names=tuple(all_in_names),
            out_names=tuple(out_names),
            lowering_input_output_aliases=(),
            sim_require_finite=False,
            sim_require_nnan=False,
            nc=nc,
        ))

    devices = jax.devices()[:B]
    mesh = Mesh(_np.asarray(devices), ("core",))
    donate = tuple(range(n_params, n_params + n_outs))
    sharded = jax.jit(
        shard_map(_body, mesh=mesh,
                  in_specs=(PartitionSpec("core"),) * (n_params + n_outs),
                  out_specs=(PartitionSpec("core"),) * n_outs,
                  check_rep=False),
        donate_argnums=donate, keep_unused=True)

    def run(in_maps):
        concat_in = [
            np.concatenate([np.asarray(in_maps[c][nm]) for c in range(B)],
                           axis=0)
            for nm in in_names
        ]
        concat_zeros = [
            np.zeros((B * a.shape[0], *a.shape[1:]), a.dtype)
            for a in out_avals
        ]
        outs = sharded(*concat_in, *concat_zeros)
        return [
            {nm: np.asarray(outs[i]).reshape(B, *out_avals[i].shape)[c]
             for i, nm in enumerate(out_names)}
            for c in range(B)
        ]

    _CACHE["runner"] = run
    return run


def kernel(x, w_perceive, w_update, steps):
    assert int(steps) == STEPS, f"kernel hardcodes steps={STEPS}, got {steps}"
    x = np.asarray(x, dtype=np.float32)
    w_perceive = np.asarray(w_perceive, dtype=np.float32)
    w_update = np.asarray(w_update, dtype=np.float32)
    assert x.shape == (B, C, H, W)

    wp_arr = np.ascontiguousarray(
        w_perceive.transpose(1, 2, 3, 0).reshape(C, 9, CO))
    wu_arr = np.ascontiguousarray(w_update[:, :, 0, 0].T)  # [128, 16]
    mxv = np.full((128, 1), 3.0, np.float32)
    mnv = np.full((128, 1), -3.0, np.float32)
    mxv[0::32] = 1.0
    mnv[0::32] = 0.0

    in_maps = [
        dict(x=np.ascontiguousarray(x[i]), wp=wp_arr, wu=wu_arr,
             maxv=mxv, minv=mnv)
        for i in range(B)
    ]
    res = _get_runner()(in_maps)
    return np.stack([res[i]["y"] for i in range(B)], axis=0)

